# revision 1
# baseline (speedup 1.0000x reference)
"""Trainium2 Bass kernel for windowed (sparse) cross-attention.

Computation (per batch row b of x:(B=2048, N=64, D=512)):
  q/k/v = x @ Wq/Wk/Wv, split into 8 heads of dim 64.
  13 temporal windows of size 16, stride 4 over N=64; softmax attention within
  each window; overlapping window outputs are accumulated and divided by the
  per-position window count; out = value @ Wout + bout.

Strategy (pure data parallel over 8 NeuronCores, batch-sharded):
  - Host pre-transposes the x shard to xT (D, T) and casts operands to bf16.
  - Projections produce qT/kT (inner-on-partitions) and v (tokens-on-partitions).
  - Scores S'[m,n] = k_m . q_n are computed per (2-batch x 2-head) "quad" with
    K=64 matmuls using tile_position row halves; the full 64x64 score block per
    pair is materialized (windows are diagonal 16x16 sub-blocks of it).
  - Window softmax is linear-algebra-ified: with E = exp(S*scale),
      window sums   s[w, n] = (MaskStack^T @ E)        (one matmul)
      R'[m, n] = MaskStack @ (U * 1/s)                 (one matmul)
      P'[m, n] = E * R'                                (elementwise)
    where U[w,n] = 1[n in window w] / cnt[n].  Then value^T = v^T-contracted
    AV matmuls over P' columns.  This makes the entire softmax+window-overlap
    accumulation 2 small matmuls + 3 vector ops per 2-quad unit.
  - Output projection consumes value^T chunks as stationary operands and
    produces the output in natural (token, D) layout; bias added on DVE.
"""

import sys

if "/opt/trn_rl_repo" not in sys.path:
    sys.path.insert(0, "/opt/trn_rl_repo")

import numpy as np
import ml_dtypes

import concourse.bass as bass
import concourse.tile as tile
from concourse import mybir
from concourse.bass_utils import run_bass_kernel_spmd

BF16 = mybir.dt.bfloat16
F32 = mybir.dt.float32
NP_BF16 = ml_dtypes.bfloat16

# Problem constants (hardcoded per contract)
B, N, D = 2048, 64, 512
NCORES = 8
BC = B // NCORES          # batch rows per core
T_FULL = BC * N           # tokens per core = 16384
HEADS, DH = 8, 64
WINDOW, STRIDE, NW = 16, 4, 13
SCALE = DH ** -0.5
TB = 512                  # tokens per block (8 batch rows)

# stash for test harness introspection
last_results = None


def _split_waits(nc, keep=1):
    """walrus in this toolchain supports only one embedded sync wait per
    instruction; hoist excess waits onto standalone EventSemaphore
    instructions on the same engine queue (FIFO => executes first)."""
    ctr = 0
    for f in nc.m.functions:
        for blk in f.blocks:
            il = blk.instructions
            out = []
            changed = False
            for inst in il:
                si = inst.sync_info
                if si is not None and len(si.on_wait) > keep:
                    waits = list(si.on_wait)
                    SyncInfo = type(si)
                    for w in waits[:-keep]:
                        evs = mybir.InstEventSemaphore(
                            name=f"WSPLIT-{ctr}", ins=[], outs=[]
                        )
                        ctr += 1
                        evs.engine = inst.engine
                        evs.sync_info = SyncInfo(on_wait=[w], on_update=[])
                        out.append(evs)
                    inst.sync_info = SyncInfo(
                        on_wait=waits[-keep:], on_update=list(si.on_update)
                    )
                    changed = True
                out.append(inst)
            if changed:
                il[:] = out
    return ctr


def _window_consts():
    idx = np.arange(NW)[:, None] * STRIDE + np.arange(WINDOW)[None, :]
    cnt = np.zeros(N, dtype=np.float64)
    np.add.at(cnt, idx, 1.0)
    member = np.zeros((N, NW), dtype=np.float64)  # member[m, w] = m in window w
    for w in range(NW):
        member[idx[w], w] = 1.0
    mask_s = np.zeros((128, 26), dtype=np.float64)
    mask_s[:64, :13] = member
    mask_s[64:, 13:] = member
    mask_t = mask_s.T.copy()
    u = np.zeros((26, 512), dtype=np.float64)
    for j in range(512):
        s = ((j % 256) // 64) % 2
        n = j % 64
        u[s * 13:(s + 1) * 13, j] = member[n] / cnt[n]
    return (
        mask_s.astype(NP_BF16),
        mask_t.astype(NP_BF16),
        u.astype(np.float32),
    )


def build_program(T=T_FULL):
    nc = bass.Bass()
    xt_d = nc.dram_tensor("xt", [D, T], BF16, kind="ExternalInput")
    wq_d = nc.dram_tensor("wq", [128, 4, D], BF16, kind="ExternalInput")
    wk_d = nc.dram_tensor("wk", [128, 4, D], BF16, kind="ExternalInput")
    wv_d = nc.dram_tensor("wv", [128, 4, D], BF16, kind="ExternalInput")
    wo_d = nc.dram_tensor("wo", [128, 4, D], BF16, kind="ExternalInput")
    bo_d = nc.dram_tensor("bo", [128, D], F32, kind="ExternalInput")
    ms_d = nc.dram_tensor("ms", [128, 26], BF16, kind="ExternalInput")
    mt_d = nc.dram_tensor("mt", [26, 128], BF16, kind="ExternalInput")
    u_d = nc.dram_tensor("u", [26, 512], F32, kind="ExternalInput")
    out_d = nc.dram_tensor("out", [T, D], F32, kind="ExternalOutput")

    NB = T // TB
    EXP = mybir.ActivationFunctionType.Exp

    with tile.TileContext(nc) as tc:
        with (
            tc.tile_pool(name="consts", bufs=1) as consts,
            tc.tile_pool(name="xtp", bufs=8) as xt_pool,
            tc.tile_pool(name="qkp", bufs=16) as qk_pool,
            tc.tile_pool(name="vp", bufs=8) as v_pool,
            tc.tile_pool(name="ep", bufs=4) as e_pool,
            tc.tile_pool(name="rcp", bufs=4) as rc_pool,
            tc.tile_pool(name="pp", bufs=4) as p_pool,
            tc.tile_pool(name="vtp", bufs=8) as vt_pool,
            tc.tile_pool(name="op", bufs=4) as out_pool,
            tc.tile_pool(name="ps_proj", bufs=2, space="PSUM") as ps_proj,
            tc.tile_pool(name="ps_s", bufs=2, space="PSUM") as ps_s,
            tc.tile_pool(name="ps_w", bufs=1, space="PSUM") as ps_w,
            tc.tile_pool(name="ps_r", bufs=1, space="PSUM") as ps_r,
            tc.tile_pool(name="ps_av", bufs=2, space="PSUM") as ps_av,
        ):
            wq_t = consts.tile([128, 4, D], BF16, tag="wq")
            nc.sync.dma_start(wq_t[:], wq_d[:])
            wk_t = consts.tile([128, 4, D], BF16, tag="wk")
            nc.sync.dma_start(wk_t[:], wk_d[:])
            wv_t = consts.tile([128, 4, D], BF16, tag="wv")
            nc.sync.dma_start(wv_t[:], wv_d[:])
            wo_t = consts.tile([128, 4, D], BF16, tag="wo")
            nc.sync.dma_start(wo_t[:], wo_d[:])
            bo_t = consts.tile([128, D], F32, tag="bo")
            nc.sync.dma_start(bo_t[:], bo_d[:])
            ms_t = consts.tile([128, 26], BF16, tag="ms")
            nc.sync.dma_start(ms_t[:], ms_d[:])
            mt_t = consts.tile([26, 128], BF16, tag="mt")
            nc.sync.dma_start(mt_t[:], mt_d[:])
            u_t = consts.tile([26, 512], F32, tag="u")
            nc.sync.dma_start(u_t[:], u_d[:])

            for blk in range(NB):
                t0 = blk * TB

                # ---- load xT tiles (D on partitions, 4 chunks) ----
                xts = []
                for kc in range(4):
                    xt_t = xt_pool.tile([128, TB], BF16, tag="xt")
                    nc.sync.dma_start(
                        xt_t[:], xt_d[kc * 128:(kc + 1) * 128, t0:t0 + TB]
                    )
                    xts.append(xt_t)

                # ---- qT / kT projections, stored as per-head-half tiles
                # [64, TB] at base partition 0 (avoids partition-offset
                # matmul operands, which wedge this hardware) ----
                qts, kts = [], []
                for wt, lst in ((wq_t, qts), (wk_t, kts)):
                    for c in range(4):
                        ps = ps_proj.tile([128, TB], F32, tag="pp")
                        for kc in range(4):
                            nc.tensor.matmul(
                                ps[:],
                                wt[:, kc, c * 128:(c + 1) * 128],
                                xts[kc][:],
                                start=(kc == 0),
                                stop=(kc == 3),
                            )
                        halves = []
                        for hh in range(2):
                            sb = qk_pool.tile([64, TB], BF16, tag="qk")
                            nc.scalar.copy(sb[:], ps[hh * 64:(hh + 1) * 64, :])
                            halves.append(sb)
                        lst.append(halves)

                # ---- v projection: natural layout [128 tokens, 512 i] ----
                vts = []
                for tt in range(4):
                    ps = ps_proj.tile([128, 512], F32, tag="pp")
                    for kc in range(4):
                        nc.tensor.matmul(
                            ps[:],
                            xts[kc][:, tt * 128:(tt + 1) * 128],
                            wv_t[:, kc, :],
                            start=(kc == 0),
                            stop=(kc == 3),
                        )
                    sb = v_pool.tile([128, 512], BF16, tag="vv")
                    nc.vector.tensor_copy(sb[:], ps[:])
                    vts.append(sb)

                # ---- attention per chunk (2 heads) ----
                vt_out = []
                for c in range(4):
                    qc, kc_t = qts[c], kts[c]
                    av = ps_av.tile([128, 512], F32, tag="av")
                    for tb2 in range(2):
                        # unit: 2 quads (each quad = 2 batch rows x 2 heads)
                        sp = ps_s.tile([128, 512], F32, tag="sp")
                        for qd in range(2):
                            tb = tb2 * 2 + qd
                            for hh in range(2):
                                tcols = slice(tb * 128, (tb + 1) * 128)
                                o = sp[:, qd * 256 + hh * 128:
                                       qd * 256 + (hh + 1) * 128]
                                nc.tensor.matmul(
                                    o, kc_t[hh][:, tcols], qc[hh][:, tcols],
                                    start=True, stop=True,
                                )
                        eu = e_pool.tile([128, 512], BF16, tag="eu")
                        nc.scalar.activation(eu[:], sp[:], EXP, scale=float(SCALE))
                        # window sums for all 4 pairs: [26, 512]
                        sw = ps_w.tile([128, 512], F32, tag="sw")
                        nc.tensor.matmul(sw[:26, :], ms_t[:], eu[:], start=True, stop=True)
                        rc = rc_pool.tile([26, 512], F32, tag="rc")
                        nc.vector.reciprocal(rc[:], sw[:26, :])
                        rcu = rc_pool.tile([26, 512], BF16, tag="rcu")
                        nc.vector.tensor_mul(rcu[:], rc[:], u_t[:])
                        rp = ps_r.tile([128, 512], F32, tag="rp")
                        nc.tensor.matmul(rp[:], mt_t[:], rcu[:], start=True, stop=True)
                        pu = p_pool.tile([128, 512], BF16, tag="pu")
                        nc.vector.tensor_mul(pu[:], eu[:], rp[:])
                        # AV: value^T quad blocks -> av[:, tb*128 + ...]
                        for qd in range(2):
                            tb = tb2 * 2 + qd
                            for hh in range(2):
                                lhsT = vts[tb][
                                    :, c * 128 + hh * 64: c * 128 + hh * 64 + 64
                                ]
                                rhs = pu[:, qd * 256 + hh * 128:
                                         qd * 256 + (hh + 1) * 128]
                                o = av[hh * 64:(hh + 1) * 64,
                                       tb * 128:(tb + 1) * 128]
                                nc.tensor.matmul(o, lhsT, rhs, start=True, stop=True)
                    vt = vt_pool.tile([128, 512], BF16, tag="vt")
                    nc.scalar.copy(vt[:], av[:])
                    vt_out.append(vt)

                # ---- output projection + bias ----
                for tt in range(4):
                    ps = ps_proj.tile([128, 512], F32, tag="pp")
                    for c in range(4):
                        nc.tensor.matmul(
                            ps[:],
                            vt_out[c][:, tt * 128:(tt + 1) * 128],
                            wo_t[:, c, :],
                            start=(c == 0),
                            stop=(c == 3),
                        )
                    ob = out_pool.tile([128, 512], F32, tag="ob")
                    nc.vector.tensor_add(ob[:], ps[:], bo_t[:])
                    nc.sync.dma_start(
                        out_d[t0 + tt * 128: t0 + (tt + 1) * 128, :], ob[:]
                    )
    return nc


def _prep_shared(Wq, Wk, Wv, Wout, bout):
    def warr(w):
        return np.ascontiguousarray(
            w.astype(np.float32).reshape(4, 128, D).transpose(1, 0, 2)
        ).astype(NP_BF16)

    mask_s, mask_t, u = _window_consts()
    return {
        "wq": warr(Wq),
        "wk": warr(Wk),
        "wv": warr(Wv),
        "wo": warr(Wout),
        "bo": np.ascontiguousarray(
            np.broadcast_to(bout.astype(np.float32), (128, D))
        ),
        "ms": mask_s,
        "mt": mask_t,
        "u": u,
    }


def kernel(x, Wq, Wk, Wv, Wout, bout):
    global last_results
    x = np.asarray(x, dtype=np.float32)
    shared = _prep_shared(
        np.asarray(Wq), np.asarray(Wk), np.asarray(Wv),
        np.asarray(Wout), np.asarray(bout),
    )
    in_maps = []
    for ci in range(NCORES):
        xs = x[ci * BC:(ci + 1) * BC].reshape(T_FULL, D)
        xt = np.ascontiguousarray(xs.T).astype(NP_BF16)
        in_maps.append({"xt": xt, **shared})

    nc = build_program(T_FULL)
    _split_waits(nc)
    res = run_bass_kernel_spmd(nc, in_maps, list(range(NCORES)))
    last_results = res
    outs = [
        res.results[ci]["out"].astype(np.float32).reshape(BC, N, D)
        for ci in range(NCORES)
    ]
    return np.concatenate(outs, axis=0)



# revision 3
# speedup vs baseline: 2.9676x; 2.9676x over previous
"""Trainium2 Bass kernel for windowed (sparse) cross-attention.

Computation (per batch row b of x:(B=2048, N=64, D=512)):
  q/k/v = x @ Wq/Wk/Wv, split into 8 heads of dim 64.
  13 temporal windows of size 16, stride 4 over N=64; softmax attention within
  each window; overlapping window outputs are accumulated and divided by the
  per-position window count; out = value @ Wout + bout.

Strategy (pure data parallel over 8 NeuronCores, batch-sharded):
  - Host pre-transposes the x shard to xT (D, T) and casts operands to bf16.
  - Per block of 512 tokens x 8 heads, attention runs as 8 (chunk c, head hh)
    "units" of [128 keys, 512 query-cols].  Scores for the hh=0/hh=1 heads of
    a chunk go to different PSUM banks and different PE row groups (operands
    at partition 0/64), so each pair of K=64 matmuls runs concurrently.
  - Window softmax is linear-algebra-ified: with E = exp(S*scale),
      window sums  s = MaskStack^T @ E  (4 units col-packed into ONE PSUM
      bank at partition offsets 0/32/64/96 via tile_position),
      1/s = exp(-ln(s)) on the scalar engine (keeps DVE free; the native DVE
      reciprocal at 26 lanes was the old kernel's bottleneck),
      rcu = (1/s) * U   (one DVE mul for 4 units),
      R = MaskStack @ rcu  (row-packed K=26 matmuls at offsets 0/32/64/96),
      P = E * R  (DVE).
  - AV matmuls (M=64) col-pack the hh pair into one PSUM bank (row halves).
  - Emission is software-pipelined 2 blocks deep: projections and the
    out-projection of the previous block fill the PE queue while the softmax
    chain (ACT/DVE) of the current block is in flight, so the tensor engine
    never idles long enough for the HAM clock gate to re-throttle.
"""

import sys

if "/opt/trn_rl_repo" not in sys.path:
    sys.path.insert(0, "/opt/trn_rl_repo")

import numpy as np
import ml_dtypes

import concourse.bass as bass
import concourse.tile as tile
from concourse import mybir
from concourse.bass_utils import run_bass_kernel_spmd

BF16 = mybir.dt.bfloat16
F32 = mybir.dt.float32
NP_BF16 = ml_dtypes.bfloat16

# Problem constants (hardcoded per contract)
B, N, D = 2048, 64, 512
NCORES = 8
BC = B // NCORES          # batch rows per core
T_FULL = BC * N           # tokens per core = 16384
HEADS, DH = 8, 64
WINDOW, STRIDE, NW = 16, 4, 13
SCALE = DH ** -0.5
TB = 512                  # tokens per block (8 batch rows)

EXP = mybir.ActivationFunctionType.Exp
LN = mybir.ActivationFunctionType.Ln

# stash for test harness introspection
last_results = None


def _split_waits(nc, keep=1):
    """walrus in this toolchain supports only one embedded sync wait per
    instruction; hoist excess waits onto standalone EventSemaphore
    instructions on the same engine queue (FIFO => executes first)."""
    ctr = 0
    for f in nc.m.functions:
        for blk in f.blocks:
            il = blk.instructions
            out = []
            changed = False
            for inst in il:
                si = inst.sync_info
                if si is not None and len(si.on_wait) > keep:
                    waits = list(si.on_wait)
                    SyncInfo = type(si)
                    for w in waits[:-keep]:
                        evs = mybir.InstEventSemaphore(
                            name=f"WSPLIT-{ctr}", ins=[], outs=[]
                        )
                        ctr += 1
                        evs.engine = inst.engine
                        evs.sync_info = SyncInfo(on_wait=[w], on_update=[])
                        out.append(evs)
                    inst.sync_info = SyncInfo(
                        on_wait=waits[-keep:], on_update=list(si.on_update)
                    )
                    changed = True
                out.append(inst)
            if changed:
                il[:] = out
    return ctr


def _window_consts():
    idx = np.arange(NW)[:, None] * STRIDE + np.arange(WINDOW)[None, :]
    cnt = np.zeros(N, dtype=np.float64)
    np.add.at(cnt, idx, 1.0)
    member = np.zeros((N, NW), dtype=np.float64)  # member[m, w] = m in window w
    for w in range(NW):
        member[idx[w], w] = 1.0
    # ms: [128 keys, 26] key->window membership (13 windows per batch subrow)
    mask_s = np.zeros((128, 26), dtype=np.float64)
    mask_s[:64, :13] = member
    mask_s[64:, 13:] = member
    # mt4: mask_s.T replicated at partition offsets 0/32/64/96
    mt4 = np.zeros((128, 128), dtype=np.float64)
    for u in range(4):
        mt4[32 * u: 32 * u + 26, :] = mask_s.T
    # u4: per-column window gather weights replicated at the 4 offsets
    u_mat = np.zeros((26, 512), dtype=np.float64)
    for j in range(512):
        sub = (j // 64) % 2
        n = j % 64
        u_mat[sub * 13:(sub + 1) * 13, j] = member[n] / cnt[n]
    u4 = np.zeros((128, 512), dtype=np.float64)
    for u in range(4):
        u4[32 * u: 32 * u + 26, :] = u_mat
    return (
        mask_s.astype(NP_BF16),
        mt4.astype(NP_BF16),
        u4.astype(np.float32),
    )


def build_program(T=T_FULL):
    nc = bass.Bass()
    xt_d = nc.dram_tensor("xt", [D, T], BF16, kind="ExternalInput")
    wq_d = nc.dram_tensor("wq", [128, 4, D], BF16, kind="ExternalInput")
    wk_d = nc.dram_tensor("wk", [128, 4, D], BF16, kind="ExternalInput")
    wv_d = nc.dram_tensor("wv", [128, 4, D], BF16, kind="ExternalInput")
    wo_d = nc.dram_tensor("wo", [128, 4, D], BF16, kind="ExternalInput")
    bo_d = nc.dram_tensor("bo", [128, D], F32, kind="ExternalInput")
    ms_d = nc.dram_tensor("ms", [128, 26], BF16, kind="ExternalInput")
    mt4_d = nc.dram_tensor("mt4", [128, 128], BF16, kind="ExternalInput")
    u4_d = nc.dram_tensor("u4", [128, 512], F32, kind="ExternalInput")
    out_d = nc.dram_tensor("out", [T, D], F32, kind="ExternalOutput")

    NB = T // TB

    with tile.TileContext(nc) as tc:
        with (
            tc.tile_pool(name="consts", bufs=1) as consts,
            tc.tile_pool(name="xtp", bufs=2) as xt_pool,
            tc.tile_pool(name="qkp", bufs=2) as qk_pool,
            tc.tile_pool(name="vp", bufs=2) as v_pool,
            tc.tile_pool(name="eup", bufs=2) as eu_pool,
            tc.tile_pool(name="lnp", bufs=2) as ln_pool,
            tc.tile_pool(name="rcup", bufs=2) as rcu_pool,
            tc.tile_pool(name="pup", bufs=2) as pu_pool,
            tc.tile_pool(name="vtp", bufs=2) as vt_pool,
            tc.tile_pool(name="op", bufs=2) as out_pool,
            tc.tile_pool(name="ps_proj", bufs=2, space="PSUM") as ps_proj,
            tc.tile_pool(name="ps_s", bufs=1, space="PSUM") as ps_s,
            tc.tile_pool(name="ps_w", bufs=1, space="PSUM") as ps_w,
            tc.tile_pool(name="ps_rp", bufs=2, space="PSUM") as ps_rp,
            tc.tile_pool(name="ps_av", bufs=1, space="PSUM") as ps_av,
        ):
            wq_t = consts.tile([128, 4, D], BF16, tag="wq")
            nc.sync.dma_start(wq_t[:], wq_d[:])
            wk_t = consts.tile([128, 4, D], BF16, tag="wk")
            nc.sync.dma_start(wk_t[:], wk_d[:])
            wv_t = consts.tile([128, 4, D], BF16, tag="wv")
            nc.sync.dma_start(wv_t[:], wv_d[:])
            wo_t = consts.tile([128, 4, D], BF16, tag="wo")
            nc.sync.dma_start(wo_t[:], wo_d[:])
            bo_t = consts.tile([128, D], F32, tag="bo")
            nc.sync.dma_start(bo_t[:], bo_d[:])
            ms_t = consts.tile([128, 26], BF16, tag="ms")
            nc.sync.dma_start(ms_t[:], ms_d[:])
            mt4_t = consts.tile([128, 128], BF16, tag="mt4")
            nc.sync.dma_start(mt4_t[:], mt4_d[:])
            u4_t = consts.tile([128, 512], F32, tag="u4")
            nc.sync.dma_start(u4_t[:], u4_d[:])

            # ---------------- per-block stage emitters ----------------

            def load_x(b):
                t0 = b * TB
                xts = []
                for kc in range(4):
                    xt_t = xt_pool.tile([128, TB], BF16, tag=f"xt{kc}")
                    nc.sync.dma_start(
                        xt_t[:], xt_d[kc * 128:(kc + 1) * 128, t0:t0 + TB]
                    )
                    xts.append(xt_t)
                return xts

            def qkproj(st):
                xts = st["xts"]
                qt, kt = [], []
                for c in range(4):
                    cs = slice(c * 128, (c + 1) * 128)
                    ps = ps_proj.tile([128, TB], F32, tag="pp")
                    for kc in range(4):
                        nc.tensor.matmul(
                            ps[:], wq_t[:, kc, cs], xts[kc][:],
                            start=(kc == 0), stop=(kc == 3),
                        )
                    q_sb = qk_pool.tile([128, TB], BF16, tag=f"qt{c}")
                    nc.scalar.copy(q_sb[:], ps[:])
                    qt.append(q_sb)
                    ps = ps_proj.tile([128, TB], F32, tag="pp")
                    for kc in range(4):
                        nc.tensor.matmul(
                            ps[:], wk_t[:, kc, cs], xts[kc][:],
                            start=(kc == 0), stop=(kc == 3),
                        )
                    k_sb = qk_pool.tile([128, TB], BF16, tag=f"kt{c}")
                    nc.vector.tensor_copy(k_sb[:], ps[:])
                    kt.append(k_sb)
                st["qt"], st["kt"] = qt, kt

            def vproj(st, tt):
                xts = st["xts"]
                ps = ps_proj.tile([128, 512], F32, tag="pp")
                for kc in range(4):
                    nc.tensor.matmul(
                        ps[:], xts[kc][:, tt * 128:(tt + 1) * 128],
                        wv_t[:, kc, :],
                        start=(kc == 0), stop=(kc == 3),
                    )
                sb = v_pool.tile([128, 512], BF16, tag=f"v{tt}")
                nc.vector.tensor_copy(sb[:], ps[:])
                st["vts"][tt] = sb

            def scores(st, c):
                """8 MMs for chunk c: hh=0 -> sp0 (rows 0-63), hh=1 -> sp1
                (rows 64-127), emitted pairwise for row-group concurrency."""
                qt, kt = st["qt"][c], st["kt"][c]
                sp0 = ps_s.tile([128, 512], F32, tag="sp0")
                sp1 = ps_s.tile([128, 512], F32, tag="sp1")
                for tb2 in range(2):
                    for qd in range(2):
                        tb = tb2 * 2 + qd
                        tcols = slice(tb * 128, (tb + 1) * 128)
                        oc = slice(tb2 * 256 + qd * 128,
                                   tb2 * 256 + (qd + 1) * 128)
                        nc.tensor.matmul(
                            sp0[:, oc], kt[0:64, tcols], qt[0:64, tcols],
                            start=True, stop=True,
                        )
                        nc.tensor.matmul(
                            sp1[:, oc], kt[64:128, tcols], qt[64:128, tcols],
                            start=True, stop=True,
                        )
                e0 = eu_pool.tile([128, 512], BF16, tag=f"eu{c}0")
                nc.scalar.activation(e0[:], sp0[:], EXP, scale=float(SCALE))
                e1 = eu_pool.tile([128, 512], BF16, tag=f"eu{c}1")
                nc.scalar.activation(e1[:], sp1[:], EXP, scale=float(SCALE))
                st["eu"][c] = (e0, e1)

            def sw(st, g):
                """4 col-packed window-sum MMs for units (c in {2g,2g+1}, hh)."""
                swb = ps_w.tile([128, 512], F32, tag="sw")
                for u in range(4):
                    c = 2 * g + u // 2
                    hh = u % 2
                    nc.tensor.matmul(
                        swb[32 * u: 32 * u + 26, :],
                        ms_t[:, 0:26], st["eu"][c][hh][:],
                        start=True, stop=True,
                        tile_position=(0, 32 * u),
                    )
                st["swb"][g] = swb

            def recip(st, g):
                """rcu = exp(-ln(s)) * u4 for the 4 packed units of group g."""
                ln_t = ln_pool.tile([128, 512], F32, tag="ln")
                nc.scalar.activation(ln_t[:], st["swb"][g][:], LN)
                rcp = ln_pool.tile([128, 512], F32, tag="rcp")
                nc.scalar.activation(rcp[:], ln_t[:], EXP, scale=-1.0)
                rcu = rcu_pool.tile([128, 512], BF16, tag=f"rcu{g}")
                nc.vector.tensor_mul(rcu[:], rcp[:], u4_t[:])
                st["rcu"][g] = rcu

            def rp_pu(st, c):
                """Row-packed R matmul pair + P = E*R for chunk c."""
                g = c // 2
                rcu = st["rcu"][g]
                for hh in range(2):
                    u = (c % 2) * 2 + hh
                    rp_t = ps_rp.tile([128, 512], F32, tag="rp")
                    nc.tensor.matmul(
                        rp_t[:], mt4_t[32 * u: 32 * u + 26, :],
                        rcu[32 * u: 32 * u + 26, :],
                        start=True, stop=True,
                        tile_position=(32 * u, 0),
                    )
                    pu_t = pu_pool.tile([128, 512], BF16, tag=f"pu{c}{hh}")
                    nc.vector.tensor_mul(
                        pu_t[:], st["eu"][c][hh][:], rp_t[:]
                    )
                    st["pu"][c][hh] = pu_t

            def av(st, c):
                """8 AV MMs for chunk c (hh col-pair concurrency), with the
                value^T copy split in halves for earlier PSUM drain."""
                vts = st["vts"]
                avb = ps_av.tile([128, 512], F32, tag="av")
                vt_sb = vt_pool.tile([128, 512], BF16, tag=f"vt{c}")
                for tb2 in range(2):
                    for qd in range(2):
                        tb = tb2 * 2 + qd
                        pc = slice(tb2 * 256 + qd * 128,
                                   tb2 * 256 + (qd + 1) * 128)
                        for hh in range(2):
                            lhsT = vts[tb][
                                :, c * 128 + hh * 64: c * 128 + hh * 64 + 64
                            ]
                            o = avb[hh * 64:(hh + 1) * 64,
                                    tb * 128:(tb + 1) * 128]
                            nc.tensor.matmul(
                                o, lhsT, st["pu"][c][hh][:, pc],
                                start=True, stop=True,
                            )
                    half = slice(tb2 * 256, (tb2 + 1) * 256)
                    nc.scalar.copy(vt_sb[:, half], avb[:, half])
                st["vt"][c] = vt_sb

            def outproj(st):
                b = st["b"]
                t0 = b * TB
                for tt in range(4):
                    ps = ps_proj.tile([128, 512], F32, tag="pp")
                    for c in range(4):
                        nc.tensor.matmul(
                            ps[:],
                            st["vt"][c][:, tt * 128:(tt + 1) * 128],
                            wo_t[:, c, :],
                            start=(c == 0), stop=(c == 3),
                        )
                    ob = out_pool.tile([128, 512], F32, tag=f"ob{tt}")
                    nc.vector.tensor_add(ob[:], ps[:], bo_t[:])
                    nc.sync.dma_start(
                        out_d[t0 + tt * 128: t0 + (tt + 1) * 128, :], ob[:]
                    )

            # ---------------- software-pipelined emission ----------------
            prev = None
            for b in range(NB):
                st = {
                    "b": b, "xts": load_x(b), "vts": [None] * 4,
                    "eu": [None] * 4, "swb": [None] * 2, "rcu": [None] * 2,
                    "pu": [[None] * 2 for _ in range(4)], "vt": [None] * 4,
                }
                qkproj(st)
                if prev is not None:
                    # tail of previous block: remaining AV chunks + rp/pu
                    av(prev, 1)
                    rp_pu(prev, 2)
                    av(prev, 2)
                    rp_pu(prev, 3)
                    av(prev, 3)
                scores(st, 0)
                vproj(st, 0)
                vproj(st, 1)
                scores(st, 1)
                vproj(st, 2)
                vproj(st, 3)
                sw(st, 0)
                recip(st, 0)
                scores(st, 2)
                if prev is not None:
                    outproj(prev)
                scores(st, 3)
                rp_pu(st, 0)
                rp_pu(st, 1)
                sw(st, 1)
                recip(st, 1)
                av(st, 0)
                prev = st

            # drain the last block
            av(prev, 1)
            rp_pu(prev, 2)
            av(prev, 2)
            rp_pu(prev, 3)
            av(prev, 3)
            outproj(prev)
    return nc


def _prep_shared(Wq, Wk, Wv, Wout, bout):
    def warr(w):
        return np.ascontiguousarray(
            w.astype(np.float32).reshape(4, 128, D).transpose(1, 0, 2)
        ).astype(NP_BF16)

    mask_s, mt4, u4 = _window_consts()
    return {
        "wq": warr(Wq),
        "wk": warr(Wk),
        "wv": warr(Wv),
        "wo": warr(Wout),
        "bo": np.ascontiguousarray(
            np.broadcast_to(bout.astype(np.float32), (128, D))
        ),
        "ms": mask_s,
        "mt4": mt4,
        "u4": u4,
    }


def kernel(x, Wq, Wk, Wv, Wout, bout):
    global last_results
    x = np.asarray(x, dtype=np.float32)
    shared = _prep_shared(
        np.asarray(Wq), np.asarray(Wk), np.asarray(Wv),
        np.asarray(Wout), np.asarray(bout),
    )
    in_maps = []
    for ci in range(NCORES):
        xs = x[ci * BC:(ci + 1) * BC].reshape(T_FULL, D)
        xt = np.ascontiguousarray(xs.T).astype(NP_BF16)
        in_maps.append({"xt": xt, **shared})

    nc = build_program(T_FULL)
    _split_waits(nc)
    res = run_bass_kernel_spmd(nc, in_maps, list(range(NCORES)))
    last_results = res
    outs = [
        res.results[ci]["out"].astype(np.float32).reshape(BC, N, D)
        for ci in range(NCORES)
    ]
    return np.concatenate(outs, axis=0)
